# revision 22
# baseline (speedup 1.0000x reference)
"""Trainium2 Bass kernel for nn_CSA (windowed conv-sparse-attention module).

Per-sample pipeline (B=8 -> 1 sample per NeuronCore, data-parallel):
  slab    = padded x rows in SBUF (bf16), window views via strided APs
  pooled  = avgpool2x2(x) from slab tap views              # Pool adds
  a       = attn_w @ pooled + attn_b                       # PE bf16
  A2      = softmax over q, written twice (dup pairs)      # ACT exp + DVE
  u_q     = Wq @ x_win_q (lhsT = slab view, no im2col)     # PE bf16
  tt      = A2[l,h,p,q] * u[l,(q,h,d)]                     # DVE bf16 2x
  parity-fold: q-summed transposes accumulate straight into
     4 output-parity PSUM accumulators (rhs = I or shift S)  # PE
  grid    = PSUM parity classes copied/added into 66x66 bf16 grid
  y       = proj_w @ grid + proj_b                         # PE bf16

Execution path: custom shard_map jit (cached), x/y shipped bf16,
weights device-cached and replicated, donated output buffers created
device-side (or recycled from the previous call).
"""

import os
import sys

import numpy as np

os.environ.setdefault("JAX_PLATFORMS", "axon,cpu")
sys.path.insert(0, "/opt/trn_rl_repo")

import concourse.bass as bass  # noqa: E402
from concourse import bacc  # noqa: E402
import concourse.tile as tile  # noqa: E402
from concourse import mybir  # noqa: E402
from concourse.masks import make_identity  # noqa: E402

F32 = mybir.dt.float32
BF16 = mybir.dt.bfloat16

AF = mybir.ActivationFunctionType
ALU = mybir.AluOpType
AX = mybir.AxisListType

K, P, S, HEADS = 3, 1, 2, 12
B, C, H, W = 8, 384, 64, 64
HD = C // HEADS          # 32
K2 = K * K               # 9
N_ATT = K2 * K2 * HEADS  # 972
L = (H // S) * (W // S)  # 1024
NCORES = 8
NLC = 8                  # l-chunks of 128 windows (4 window-rows each)
CCH = C // 128           # 3 channel chunks
HCH = HEADS // CCH       # 4 heads per channel chunk
GW = W + 2               # padded grid width 66
GH = 10                  # slab rows per l-chunk: x rows [8*lc-1, 8*lc+9)


def _tap_ap(tile_ap, qi, qj):
    """[c, (i,j)] window view of a [128, GH*GW] slab for kernel tap
    (qi, qj): element (2i+qi)*GW + (2j+qj)."""
    return bass.AP(
        tile_ap.tensor, tile_ap.offset + qi * GW + qj,
        [list(tile_ap.ap[0])] + [[2 * GW, 4], [2, 32]],
    )


def _build(dbg=False):
    nc = bacc.Bacc("TRN2", target_bir_lowering=False, debug=False)

    x_d = nc.declare_dram_parameter("x", [C, H * W], BF16, isOutput=False)
    aw_d = nc.declare_dram_parameter("aw", [C, N_ATT], BF16, isOutput=False)
    ab_d = nc.declare_dram_parameter("ab", [1, N_ATT], BF16, isOutput=False)
    cw_d = nc.declare_dram_parameter("cw", [C, K2 * C], BF16, isOutput=False)
    pw_d = nc.declare_dram_parameter("pw", [C, C], BF16, isOutput=False)
    pb_d = nc.declare_dram_parameter("pb", [C, 1], F32, isOutput=False)
    y_d = nc.declare_dram_parameter("y", [C, H * W], BF16, isOutput=True)

    xg = x_d.ap().rearrange("c (h w) -> c h w", h=H)

    with tile.TileContext(nc) as tc:
        with (
            tc.tile_pool(name="wts", bufs=1) as wpool,
            tc.tile_pool(name="small", bufs=2) as spool,
            tc.tile_pool(name="ubuf", bufs=2) as upool,
            tc.tile_pool(name="tbuf", bufs=6) as tpool,
            tc.tile_pool(name="obuf", bufs=4) as opool,
            tc.tile_pool(name="acc", bufs=1) as accpool,
            tc.tile_pool(name="ps_a", bufs=1, space="PSUM") as ps_a,
            tc.tile_pool(name="ps_u", bufs=2, space="PSUM") as ps_u,
            tc.tile_pool(name="ps_c", bufs=1, space="PSUM") as ps_c,
        ):
            # double-buffered padded x slabs (bf16): zero + load lc0 FIRST
            # so compute isn't queued behind 3.4MB of weight DMA; the lc+1
            # loads overlap lc's conv reads
            xts2 = [[accpool.tile([128, GH * GW], BF16, tag=f"xt{s}{k}",
                                  name=f"xt{s}{k}") for k in range(CCH)]
                    for s in range(2)]
            for s in range(2):
                for k in range(CCH):
                    nc.gpsimd.memset(xts2[s][k][:], 0.0)

            def _slab_load(lc):
                # windows of chunk lc touch x rows [8lc-1, 8lc+8); slab row
                # 9 is never read (max tap row index is 2*3+2 = 8)
                xr0 = 8 * lc - 1
                r_lo, r_hi = max(0, xr0), min(H, 8 * lc + 8)
                for k in range(CCH):
                    dst = xts2[lc % 2][k][:].rearrange("p (h w) -> p h w", h=GH)
                    nc.sync.dma_start(
                        out=dst[:, r_lo - xr0:r_hi - xr0, 1:W + 1],
                        in_=xg[k * 128:(k + 1) * 128, r_lo:r_hi, :],
                    )

            _slab_load(0)

            # ---- persistent weights in SBUF ----
            aw_sb = [wpool.tile([128, N_ATT], BF16, tag=f"aw{k}", name=f"aw{k}")
                     for k in range(CCH)]
            cw_sb = [wpool.tile([128, K2 * C], BF16, tag=f"cw{k}", name=f"cw{k}")
                     for k in range(CCH)]
            pw_sb = [wpool.tile([128, C], BF16, tag=f"pw{k}", name=f"pw{k}")
                     for k in range(CCH)]
            ab_sb = wpool.tile([1, N_ATT], BF16, tag="ab")
            pb_sb = wpool.tile([128, CCH], F32, tag="pb")
            ones_l32 = wpool.tile([1, 128], F32, tag="ones_l32")
            ones_l = wpool.tile([1, 128], BF16, tag="ones_l")
            ident = wpool.tile([128, 128], F32, tag="ident")
            ident_bf = wpool.tile([128, 128], BF16, tag="ident_bf")
            # identx_bf = [I' | 0] where I' zeroes (32,32),(64,64),(96,96);
            # identx_bf[:, 1:129] = S (within-32-block down-shift): S[r,c]=1
            # iff r==c+1 and r%32!=0 -> fold x-shift without row wraparound
            identx_bf = wpool.tile([128, 256], BF16, tag="identx_bf")
            zeros_bf = wpool.tile([128, 512], BF16, tag="zeros_bf")
            for k in range(CCH):
                csl = slice(k * 128, (k + 1) * 128)
                nc.sync.dma_start(out=aw_sb[k][:], in_=aw_d.ap()[csl, :])
            nc.sync.dma_start(out=ab_sb[:], in_=ab_d.ap())
            for k in range(CCH):
                csl = slice(k * 128, (k + 1) * 128)
                nc.sync.dma_start(out=cw_sb[k][:], in_=cw_d.ap()[csl, :])
            for k in range(CCH):
                csl = slice(k * 128, (k + 1) * 128)
                nc.sync.dma_start(out=pw_sb[k][:], in_=pw_d.ap()[csl, :])
            nc.sync.dma_start(out=pb_sb[:], in_=pb_d.ap().rearrange("(m p) o -> p (m o)", m=CCH))
            nc.gpsimd.memset(ones_l32[:], 1.0)
            nc.scalar.copy(out=ones_l[:], in_=ones_l32[:])
            make_identity(nc, ident[:])
            nc.scalar.copy(out=ident_bf[:], in_=ident[:])
            nc.vector.memset(identx_bf[:], 0.0)
            nc.scalar.copy(out=identx_bf[:, 0:128], in_=ident[:])
            for r in (32, 64, 96):
                nc.vector.memset(identx_bf[:][r:r + 1, r:r + 1], 0.0)
            nc.vector.memset(zeros_bf[:], 0.0)
            shift_bf = identx_bf[:, 1:129]

            # folded output grid, padded 66x66 (bf16), one per c-chunk.
            # Even output rows/cols are written exactly once (PSUM f32
            # parity accumulators); only the odd-row chunk-boundary rows
            # are accumulated in bf16.
            grid = [accpool.tile([128, GW * GW], BF16, tag=f"gr{k}",
                                 name=f"grid{k}") for k in range(CCH)]
            for k in range(CCH):
                nc.gpsimd.memset(grid[k][:], 0.0)

            def _proj_stripe(t):
                rows = 7 if t < 9 else 1
                g0 = (1 + 7 * t) * GW
                for m in range(CCH):
                    msl = slice(m * 128, (m + 1) * 128)
                    py = ps_u.tile([128, 512], F32, tag="pu", name="py")
                    pys = py[:, :rows * GW]
                    for k in range(CCH):
                        nc.tensor.matmul(
                            pys, pw_sb[k][:, msl],
                            grid[k][:, g0:g0 + rows * GW],
                            start=(k == 0), stop=(k == CCH - 1))
                    yst = opool.tile([128, 448], BF16, tag="yst", name="yst")
                    nc.scalar.activation(
                        out=yst[:, :rows * 64].rearrange(
                            "p (r w) -> p r w", r=rows),
                        in_=py[:, :rows * GW].rearrange(
                            "p (r w) -> p r w", r=rows)[:, :, 1:W + 1],
                        func=AF.Identity, bias=pb_sb[:, m:m + 1])
                    nc.sync.dma_start(
                        out=y_d.ap()[msl, 7 * t * 64:(7 * t + rows) * 64],
                        in_=yst[:, :rows * 64])

            # tap indices for the 2x2 avgpool (0.25 folded into aw)
            POOL_TAPS = [(1, 1), (1, 2), (2, 1), (2, 2)]

            # ---- main loop over l-chunks ----
            proj_done = 0
            for lc in range(NLC):
                if lc + 1 < NLC:
                    # prefetch the next chunk's slab into the other buffer
                    _slab_load(lc + 1)

                # pooled[k] = sum of 4 center taps (c-part layout)
                pooled = [spool.tile([128, 128], BF16, tag=f"pl{k}",
                                     name=f"pl{k}") for k in range(CCH)]
                xts = xts2[lc % 2]
                for k in range(CCH):
                    pv = pooled[k][:].rearrange("p (i j) -> p i j", i=4)
                    t11 = _tap_ap(xts[k][:], 1, 1)
                    t12 = _tap_ap(xts[k][:], 1, 2)
                    t21 = _tap_ap(xts[k][:], 2, 1)
                    t22 = _tap_ap(xts[k][:], 2, 2)
                    with nc.allow_low_precision(reason="bf16 avgpool"):
                        nc.gpsimd.tensor_tensor(out=pv, in0=t11, in1=t12,
                                                op=ALU.add)
                        nc.gpsimd.tensor_tensor(out=pv, in0=pv, in1=t21,
                                                op=ALU.add)
                        nc.gpsimd.tensor_tensor(out=pv, in0=pv, in1=t22,
                                                op=ALU.add)

                # ---- attention scores: a[l, n] (n = h*81 + p*9 + q) ----
                pa0 = ps_a.tile([128, 512], F32, tag="pa0", name="pa0")
                pa1 = ps_a.tile([128, 512], F32, tag="pa1", name="pa1")
                pa_parts = [(pa0[:, :512], slice(0, 512)),
                            (pa1[:, :N_ATT - 512], slice(512, N_ATT))]
                for k in range(CCH):
                    for pap, nsl in pa_parts:
                        nc.tensor.matmul(
                            pap, pooled[k][:], aw_sb[k][:, nsl],
                            start=(k == 0), stop=False)
                for pap, nsl in pa_parts:
                    nc.tensor.matmul(pap, ones_l[:], ab_sb[:, nsl],
                                     start=False, stop=True)

                # ---- softmax over q (logits tiny, skip max-subtract) ----
                att = spool.tile([128, N_ATT], BF16, tag="att")
                nc.scalar.activation(out=att[:, :512], in_=pa0[:, :512],
                                     func=AF.Exp)
                nc.scalar.activation(out=att[:, 512:], in_=pa1[:, :N_ATT - 512],
                                     func=AF.Exp)
                den = spool.tile([128, 108], BF16, tag="den")
                denr = spool.tile([128, 108], BF16, tag="denr")
                with nc.allow_low_precision(reason="softmax denom in bf16"):
                    nc.vector.tensor_reduce(
                        out=den[:],
                        in_=att[:].rearrange("p (g q) -> p g q", q=K2),
                        axis=AX.X, op=ALU.add)
                    nc.vector.reciprocal(out=denr[:], in_=den[:])
                nc.vector.tensor_tensor(
                    out=att[:].rearrange("p (g q) -> p g q", q=K2),
                    in0=att[:].rearrange("p (g q) -> p g q", q=K2),
                    in1=denr[:].unsqueeze(2).broadcast_to([128, 108, K2]),
                    op=ALU.mult)

                # ---- grouped conv: u[l, (h, d, q)] ----
                # matmul weight APs must merge to one free dim, so the
                # (i, j) window view is fed as 4 concurrent col-tiles
                # (one per window row, tile_position on 32-col groups)
                u_sb = upool.tile([128, K2 * C], BF16, tag="u")
                uq = u_sb[:].rearrange("p (h d q) -> p q h d",
                                       h=HEADS, d=HD, q=K2)
                for q in range(K2):
                    qi, qj = q // K, q % K
                    pu = ps_u.tile([128, 512], F32, tag="pu", name="pu")
                    for k in range(CCH):
                        for i in range(4):
                            row = bass.AP(
                                xts[k].tensor,
                                xts[k][:].offset + (2 * i + qi) * GW + qj,
                                [list(xts[k][:].ap[0]), [2, 32]])
                            nc.tensor.matmul(
                                pu[32 * i:32 * (i + 1), :C], row,
                                cw_sb[k][:, q * C:(q + 1) * C],
                                start=(k == 0), stop=(k == CCH - 1),
                                tile_position=(0, 32 * i))
                    # interleaved (h, d, q) store: strided writes run at
                    # 1x everywhere; keep them on ACT (DVE is the
                    # critical path)
                    nc.scalar.copy(
                        out=uq[:, q],
                        in_=pu[:, :C].rearrange("p (h d) -> p h d", h=HEADS))

                # ---- apply + q-summed transpose + parity fold ----
                # parity classes: rows E (y even <- pi=1) / O (y odd <-
                # pi in {0,2}); cols E (x even <- pj=1) / O (pj in {0,2}).
                # O-row accumulators have 5 rows (160 cols); row 0 is the
                # chunk-boundary row shared with the previous lc.
                # T1 bank: ee [0:128], eo [128:256], oe [256:416]
                # T2 bank: oo [0:160]
                # A zero dummy matmul (start=True) covers each bank's full
                # used range first: clears has_written for the whole bank,
                # writes 0, and WAW-orders every later in-range matmul
                # after it. All real matmuls then accumulate (start=False).
                for k in range(CCH):
                    pc1 = ps_c.tile([128, 512], F32, tag="pc1", name="pc1")
                    pc2 = ps_c.tile([128, 512], F32, tag="pc2", name="pc2")
                    nc.tensor.matmul(pc1[:, 0:416], ident_bf[:],
                                     zeros_bf[:, 0:416], start=True, stop=False)
                    nc.tensor.matmul(pc2[:, 0:160], ident_bf[:],
                                     zeros_bf[:, 0:160], start=True, stop=False)
                    n1 = n2 = 0  # real-MM counters for stop flags

                    h0 = k * HCH
                    uv = u_sb[:].rearrange("p (h d q) -> p h d q",
                                           h=HEADS, d=HD, q=K2)
                    attv = att[:].rearrange("p (h pp q) -> p pp h q",
                                            h=HEADS, pp=K2)
                    for p in range(K2):
                        pi, pj = p // K, p % K
                        tt = tpool.tile([128, K2 * 128], BF16, tag="tt")
                        ttv = tt[:].rearrange("p (h d q) -> p h d q",
                                              h=HCH, d=HD, q=K2)
                        with nc.allow_low_precision(reason="bf16 apply"):
                            nc.vector.tensor_tensor(
                                out=ttv,
                                in0=uv[:, h0:h0 + HCH, :, :],
                                in1=attv[:, p, h0:h0 + HCH].unsqueeze(2)
                                    .broadcast_to([128, HCH, HD, K2]),
                                op=ALU.mult)

                        if pi == 1:
                            coff = 0 if pj == 1 else 128    # ee / eo
                        else:
                            # odd rows: r' = i for pi=0, i+1 for pi=2
                            coff = 256 + (32 if pi == 2 else 0)  # oe
                        use2 = (pi != 1) and (pj != 1)           # oo
                        if use2:
                            ptile, coff = pc2, (32 if pi == 2 else 0)
                        else:
                            ptile = pc1
                        rhs = shift_bf if pj == 0 else ident_bf[:]
                        ttq = tt[:].rearrange("p (h d q) -> p q h d",
                                              h=HCH, d=HD, q=K2)
                        for q in range(K2):
                            if use2:
                                n2 += 1
                                sp = n2 == 36
                            else:
                                n1 += 1
                                sp = n1 == 45
                            nc.tensor.matmul(
                                ptile[:, coff:coff + 128],
                                ttq[:, q], rhs,
                                start=False, stop=sp)

                    # ---- evacuate parity classes into the bf16 grid ----
                    gt = grid[k]
                    base_e = (8 * lc + 1) * GW   # first even row (y=8lc)
                    base_o1 = (8 * lc + 2) * GW  # odd rows y=8lc+1..
                    base_o0 = (8 * lc) * GW      # boundary row y=8lc-1

                    def _gview(off, nrows):
                        return bass.AP(gt.tensor, gt[:].offset + off,
                                       [list(gt[:].ap[0]),
                                        [2 * GW, nrows], [2, 32]])

                    nc.scalar.copy(out=_gview(base_e + 1, 4),
                                   in_=pc1[:, 0:128].rearrange(
                                       "p (r j) -> p r j", r=4))
                    nc.scalar.copy(out=_gview(base_e + 2, 4),
                                   in_=pc1[:, 128:256].rearrange(
                                       "p (r j) -> p r j", r=4))
                    nc.scalar.copy(out=_gview(base_o1 + 1, 4),
                                   in_=pc1[:, 288:416].rearrange(
                                       "p (r j) -> p r j", r=4))
                    nc.scalar.copy(out=_gview(base_o1 + 2, 4),
                                   in_=pc2[:, 32:160].rearrange(
                                       "p (r j) -> p r j", r=4))
                    with nc.allow_low_precision(reason="bf16 boundary add"):
                        nc.vector.tensor_tensor(
                            out=_gview(base_o0 + 1, 1), in0=_gview(base_o0 + 1, 1),
                            in1=pc1[:, 256:288].rearrange("p (r j) -> p r j", r=1),
                            op=ALU.add)
                        nc.vector.tensor_tensor(
                            out=_gview(base_o0 + 2, 1), in0=_gview(base_o0 + 2, 1),
                            in1=pc2[:, 0:32].rearrange("p (r j) -> p r j", r=1),
                            op=ALU.add)

                # projection stripes whose rows are now final
                ready = 10 if lc == NLC - 1 else min(9, (8 * lc) // 7) + 1
                while proj_done < ready:
                    _proj_stripe(proj_done)
                    proj_done += 1

    if not nc.is_finalized():
        nc.finalize()
    return nc


_NC_CACHE = None


def _get_nc():
    global _NC_CACHE
    if _NC_CACHE is None:
        _NC_CACHE = _build()
    return _NC_CACHE


def _bf16(a):
    import ml_dtypes
    return np.ascontiguousarray(a.astype(ml_dtypes.bfloat16))


def _prep_weights(attn_w, attn_b, conv_w, proj_w, proj_b):
    scale = (C // HEADS) ** -0.5
    aw = (attn_w.astype(np.float64) * scale * 0.25).astype(np.float32)
    aw_t = _bf16(aw.T)                                                   # [C, 972]
    ab = _bf16((attn_b * scale).astype(np.float32).reshape(1, N_ATT))
    cw = conv_w.reshape(K2, C, C).transpose(2, 0, 1).reshape(C, K2 * C)  # [c_in, (q, c_out)]
    cw = _bf16(cw.astype(np.float32))
    pw_t = _bf16(proj_w.astype(np.float32).T)                            # [c_in, c_out]
    pb = np.ascontiguousarray(proj_b.astype(np.float32).reshape(C, 1))
    return aw_t, ab, cw, pw_t, pb


_EXEC = None     # jitted shard_map executable + shardings
_WDEV = None     # (host weight arrays, device weight arrays) cache
_YBUF = None     # previous output device buffer, reused as donated output


def _make_exec(nc):
    import jax
    import jax.numpy as jnp
    from jax.experimental.shard_map import shard_map
    from jax.sharding import Mesh, NamedSharding, PartitionSpec as PSpec
    from concourse import bass2jax
    import concourse.mybir as mybir_

    bass2jax.install_neuronx_cc_hook()
    partition_name = (nc.partition_id_tensor.name
                      if nc.partition_id_tensor else None)
    in_names, out_names, out_avals = [], [], []
    for alloc in nc.m.functions[0].allocations:
        if not isinstance(alloc, mybir_.MemoryLocationSet):
            continue
        name = alloc.memorylocations[0].name
        if alloc.kind == "ExternalInput":
            if name != partition_name:
                in_names.append(name)
        elif alloc.kind == "ExternalOutput":
            out_names.append(name)
            out_avals.append(jax.core.ShapedArray(
                tuple(alloc.tensor_shape), mybir_.dt.np(alloc.dtype)))
    n_params = len(in_names)
    all_in = list(in_names) + list(out_names)
    if partition_name is not None:
        all_in.append(partition_name)

    def _body(*args):
        operands = list(args)
        if partition_name is not None:
            operands.append(bass2jax.partition_id_tensor())
        outs = bass2jax._bass_exec_p.bind(
            *operands, out_avals=tuple(out_avals), in_names=tuple(all_in),
            out_names=tuple(out_names), lowering_input_output_aliases=(),
            sim_require_finite=True, sim_require_nnan=True, nc=nc)
        return tuple(outs)

    devices = jax.devices()[:NCORES]
    mesh = Mesh(np.asarray(devices), ("core",))
    percore = {"x"}
    specs = [PSpec("core") if nm in percore else PSpec() for nm in in_names]
    specs += [PSpec("core")] * len(out_names)
    donate = tuple(range(n_params, n_params + len(out_names)))
    sharded = jax.jit(
        shard_map(_body, mesh=mesh, in_specs=tuple(specs),
                  out_specs=(PSpec("core"),) * len(out_names),
                  check_rep=False),
        donate_argnums=donate, keep_unused=True)
    zsh = NamedSharding(mesh, PSpec("core"))
    zshapes = [((NCORES * av.shape[0],) + tuple(av.shape[1:]), av.dtype)
               for av in out_avals]
    zjit = jax.jit(lambda: tuple(jnp.zeros(s, d) for s, d in zshapes),
                   out_shardings=(zsh,) * len(out_names))
    return {"sharded": sharded, "zjit": zjit,
            "xsh": NamedSharding(mesh, PSpec("core")),
            "wsh": NamedSharding(mesh, PSpec()),
            "in_names": in_names, "out_names": out_names}


def _weights_dev(wmap, E):
    """Device-put the (replicated) weights; reuse cached device arrays
    when the host contents are unchanged between calls."""
    global _WDEV
    import jax
    if _WDEV is not None:
        host, dev = _WDEV
        if all(np.array_equal(host[k], wmap[k]) for k in wmap):
            return dev
    dev = {k: jax.device_put(v, E["wsh"]) for k, v in wmap.items()}
    jax.block_until_ready(list(dev.values()))
    _WDEV = ({k: v.copy() for k, v in wmap.items()}, dev)
    return dev


def _run_fast(x_bf, wmap, E, timing=None):
    import time
    import jax
    global _YBUF
    wdev = _weights_dev(wmap, E)
    t0 = time.perf_counter()
    xdev = jax.device_put(x_bf, E["xsh"])
    jax.block_until_ready(xdev)
    t1 = time.perf_counter()
    if _YBUF is not None:
        # y is fully overwritten by the kernel; donate last call's output
        # buffer instead of dispatching a fresh device-side zeros fill.
        zeros = (_YBUF,)
        _YBUF = None
    else:
        zeros = E["zjit"]()
        jax.block_until_ready(zeros)
    inmap = {"x": xdev, **wdev}
    args = [inmap[nm] for nm in E["in_names"]]
    t2 = time.perf_counter()
    out = E["sharded"](*args, *zeros)
    jax.block_until_ready(out)
    t3 = time.perf_counter()
    y = np.asarray(out[0])
    _YBUF = out[0]
    t4 = time.perf_counter()
    if timing is not None:
        timing.append({"x_put": t1 - t0, "zeros": t2 - t1,
                       "exec": t3 - t2, "fetch": t4 - t3})
    return y


def kernel(x, attn_w, attn_b, conv_w, proj_w, proj_b, _trace=False, _dbg=False):
    global _EXEC
    x = np.asarray(x, dtype=np.float32)
    aw_t, ab, cw, pw_t, pb = _prep_weights(
        np.asarray(attn_w), np.asarray(attn_b), np.asarray(conv_w),
        np.asarray(proj_w), np.asarray(proj_b))
    wmap = {"aw": aw_t, "ab": ab, "cw": cw, "pw": pw_t, "pb": pb}
    x_bf = _bf16(x.reshape(NCORES * C, H * W))
    nc = _get_nc()
    try:
        if _EXEC is None:
            _EXEC = _make_exec(nc)
        y2d = _run_fast(x_bf, wmap, _EXEC, timing=None)
    except Exception as e:
        print(f"fast path failed ({type(e).__name__}: {e}); using legacy",
              file=sys.stderr)
        from concourse.bass_utils import run_bass_kernel_spmd
        in_maps = []
        for b in range(NCORES):
            in_maps.append({
                "x": np.ascontiguousarray(x_bf[b * C:(b + 1) * C]),
                **wmap,
            })
        res = run_bass_kernel_spmd(nc, in_maps, list(range(NCORES)),
                                   trace=False)
        y2d = np.concatenate([np.asarray(res.results[b]["y"])
                              for b in range(NCORES)], axis=0)
    return y2d.astype(np.float32).reshape(B, C, H, W)


# revision 25
# speedup vs baseline: 1.1960x; 1.1960x over previous
"""Trainium2 Bass kernel for nn_CSA (windowed conv-sparse-attention module).

Per-sample pipeline (B=8 -> 1 sample per NeuronCore, data-parallel):
  slab    = padded x rows in SBUF (bf16), window views via strided APs
  pooled  = avgpool2x2(x) from slab tap views              # Pool adds
  a       = attn_w @ pooled + attn_b                       # PE bf16
  A2      = softmax over q, written twice (dup pairs)      # ACT exp + DVE
  u_q     = Wq @ x_win_q (lhsT = slab view, no im2col)     # PE bf16
  tt      = A2[l,h,p,q] * u[l,(q,h,d)]                     # DVE bf16 2x
  parity-fold: q-summed transposes accumulate straight into
     4 output-parity PSUM accumulators (rhs = I or shift S)  # PE
  grid    = PSUM parity classes copied/added into 66x66 bf16 grid
  y       = proj_w @ grid + proj_b                         # PE bf16

Execution path: custom shard_map jit (cached), x/y shipped bf16,
weights device-cached and replicated, donated output buffers created
device-side (or recycled from the previous call).
"""

import os
import sys

import numpy as np

os.environ.setdefault("JAX_PLATFORMS", "axon,cpu")
sys.path.insert(0, "/opt/trn_rl_repo")

import concourse.bass as bass  # noqa: E402
from concourse import bacc  # noqa: E402
import concourse.tile as tile  # noqa: E402
from concourse import mybir  # noqa: E402
from concourse.masks import make_identity  # noqa: E402

F32 = mybir.dt.float32
BF16 = mybir.dt.bfloat16

AF = mybir.ActivationFunctionType
ALU = mybir.AluOpType
AX = mybir.AxisListType

K, P, S, HEADS = 3, 1, 2, 12
B, C, H, W = 8, 384, 64, 64
HD = C // HEADS          # 32
K2 = K * K               # 9
N_ATT = K2 * K2 * HEADS  # 972
L = (H // S) * (W // S)  # 1024
NCORES = 8
NLC = 8                  # l-chunks of 128 windows (4 window-rows each)
CCH = C // 128           # 3 channel chunks
HCH = HEADS // CCH       # 4 heads per channel chunk
GW = W + 2               # padded grid width 66
GH = 10                  # slab rows per l-chunk: x rows [8*lc-1, 8*lc+9)


def _tap_ap(tile_ap, qi, qj):
    """[c, (i,j)] window view of a [128, GH*GW] slab for kernel tap
    (qi, qj): element (2i+qi)*GW + (2j+qj)."""
    return bass.AP(
        tile_ap.tensor, tile_ap.offset + qi * GW + qj,
        [list(tile_ap.ap[0])] + [[2 * GW, 4], [2, 32]],
    )


def _build(dbg=False):
    nc = bacc.Bacc("TRN2", target_bir_lowering=False, debug=False)

    x_d = nc.declare_dram_parameter("x", [C, H * W], BF16, isOutput=False)
    aw_d = nc.declare_dram_parameter("aw", [C, N_ATT], BF16, isOutput=False)
    ab_d = nc.declare_dram_parameter("ab", [1, N_ATT], BF16, isOutput=False)
    cw_d = nc.declare_dram_parameter("cw", [C, K2 * C], BF16, isOutput=False)
    pw_d = nc.declare_dram_parameter("pw", [C, C], BF16, isOutput=False)
    pb_d = nc.declare_dram_parameter("pb", [C, 1], F32, isOutput=False)
    y_d = nc.declare_dram_parameter("y", [C, H * W], BF16, isOutput=True)

    xg = x_d.ap().rearrange("c (h w) -> c h w", h=H)

    with tile.TileContext(nc) as tc:
        with (
            tc.tile_pool(name="wts", bufs=1) as wpool,
            tc.tile_pool(name="small", bufs=2) as spool,
            tc.tile_pool(name="ubuf", bufs=2) as upool,
            tc.tile_pool(name="tbuf", bufs=6) as tpool,
            tc.tile_pool(name="obuf", bufs=4) as opool,
            tc.tile_pool(name="acc", bufs=1) as accpool,
            tc.tile_pool(name="ps_a", bufs=1, space="PSUM") as ps_a,
            tc.tile_pool(name="ps_u", bufs=2, space="PSUM") as ps_u,
            tc.tile_pool(name="ps_c", bufs=1, space="PSUM") as ps_c,
        ):
            # double-buffered padded x slabs (bf16): zero + load lc0 FIRST
            # so compute isn't queued behind 3.4MB of weight DMA; the lc+1
            # loads overlap lc's conv reads
            xts2 = [[accpool.tile([128, GH * GW], BF16, tag=f"xt{s}{k}",
                                  name=f"xt{s}{k}") for k in range(CCH)]
                    for s in range(2)]
            for s in range(2):
                for k in range(CCH):
                    nc.gpsimd.memset(xts2[s][k][:], 0.0)

            def _slab_load(lc):
                # windows of chunk lc touch x rows [8lc-1, 8lc+8); slab row
                # 9 is never read (max tap row index is 2*3+2 = 8)
                xr0 = 8 * lc - 1
                r_lo, r_hi = max(0, xr0), min(H, 8 * lc + 8)
                for k in range(CCH):
                    dst = xts2[lc % 2][k][:].rearrange("p (h w) -> p h w", h=GH)
                    nc.sync.dma_start(
                        out=dst[:, r_lo - xr0:r_hi - xr0, 1:W + 1],
                        in_=xg[k * 128:(k + 1) * 128, r_lo:r_hi, :],
                    )

            _slab_load(0)

            # ---- persistent weights in SBUF ----
            aw_sb = [wpool.tile([128, N_ATT], BF16, tag=f"aw{k}", name=f"aw{k}")
                     for k in range(CCH)]
            cw_sb = [wpool.tile([128, K2 * C], BF16, tag=f"cw{k}", name=f"cw{k}")
                     for k in range(CCH)]
            pw_sb = [wpool.tile([128, C], BF16, tag=f"pw{k}", name=f"pw{k}")
                     for k in range(CCH)]
            ab_sb = wpool.tile([1, N_ATT], BF16, tag="ab")
            pb_sb = wpool.tile([128, CCH], F32, tag="pb")
            ones_l32 = wpool.tile([1, 128], F32, tag="ones_l32")
            ones_l = wpool.tile([1, 128], BF16, tag="ones_l")
            ident = wpool.tile([128, 128], F32, tag="ident")
            ident_bf = wpool.tile([128, 128], BF16, tag="ident_bf")
            # identx_bf = [I' | 0] where I' zeroes (32,32),(64,64),(96,96);
            # identx_bf[:, 1:129] = S (within-32-block down-shift): S[r,c]=1
            # iff r==c+1 and r%32!=0 -> fold x-shift without row wraparound
            identx_bf = wpool.tile([128, 256], BF16, tag="identx_bf")
            zeros_bf = wpool.tile([128, 512], BF16, tag="zeros_bf")
            for k in range(CCH):
                csl = slice(k * 128, (k + 1) * 128)
                nc.sync.dma_start(out=aw_sb[k][:], in_=aw_d.ap()[csl, :])
            nc.sync.dma_start(out=ab_sb[:], in_=ab_d.ap())
            for k in range(CCH):
                csl = slice(k * 128, (k + 1) * 128)
                nc.sync.dma_start(out=cw_sb[k][:], in_=cw_d.ap()[csl, :])
            for k in range(CCH):
                csl = slice(k * 128, (k + 1) * 128)
                nc.sync.dma_start(out=pw_sb[k][:], in_=pw_d.ap()[csl, :])
            nc.sync.dma_start(out=pb_sb[:], in_=pb_d.ap().rearrange("(m p) o -> p (m o)", m=CCH))
            nc.gpsimd.memset(ones_l32[:], 1.0)
            nc.scalar.copy(out=ones_l[:], in_=ones_l32[:])
            make_identity(nc, ident[:])
            nc.scalar.copy(out=ident_bf[:], in_=ident[:])
            nc.vector.memset(identx_bf[:], 0.0)
            nc.scalar.copy(out=identx_bf[:, 0:128], in_=ident[:])
            for r in (32, 64, 96):
                nc.vector.memset(identx_bf[:][r:r + 1, r:r + 1], 0.0)
            nc.vector.memset(zeros_bf[:], 0.0)
            shift_bf = identx_bf[:, 1:129]

            # folded output grid, padded 66x66 (bf16), one per c-chunk.
            # Even output rows/cols are written exactly once (PSUM f32
            # parity accumulators); only the odd-row chunk-boundary rows
            # are accumulated in bf16.
            grid = [accpool.tile([128, GW * GW], BF16, tag=f"gr{k}",
                                 name=f"grid{k}") for k in range(CCH)]
            for k in range(CCH):
                nc.gpsimd.memset(grid[k][:], 0.0)

            def _proj_stripe(t):
                rows = 7 if t < 9 else 1
                g0 = (1 + 7 * t) * GW
                for m in range(CCH):
                    msl = slice(m * 128, (m + 1) * 128)
                    py = ps_u.tile([128, 512], F32, tag="pu", name="py")
                    pys = py[:, :rows * GW]
                    for k in range(CCH):
                        nc.tensor.matmul(
                            pys, pw_sb[k][:, msl],
                            grid[k][:, g0:g0 + rows * GW],
                            start=(k == 0), stop=(k == CCH - 1))
                    yst = opool.tile([128, 448], BF16, tag="yst", name="yst")
                    nc.scalar.activation(
                        out=yst[:, :rows * 64].rearrange(
                            "p (r w) -> p r w", r=rows),
                        in_=py[:, :rows * GW].rearrange(
                            "p (r w) -> p r w", r=rows)[:, :, 1:W + 1],
                        func=AF.Identity, bias=pb_sb[:, m:m + 1])
                    nc.sync.dma_start(
                        out=y_d.ap()[msl, 7 * t * 64:(7 * t + rows) * 64],
                        in_=yst[:, :rows * 64])

            # ---- per-chunk stages (software-pipelined: the head of
            # chunk lc+1 is emitted BEFORE the body of chunk lc so each
            # engine's in-order stream overlaps the phases) ----

            def _head(lc):
                """pooled -> attention -> softmax -> conv; returns
                (att, u_sb) for chunk lc."""
                # pooled[k] = sum of 4 center taps (c-part layout)
                pooled = [spool.tile([128, 128], BF16, tag=f"pl{k}",
                                     name=f"pl{k}") for k in range(CCH)]
                xts = xts2[lc % 2]
                for k in range(CCH):
                    pv = pooled[k][:].rearrange("p (i j) -> p i j", i=4)
                    t11 = _tap_ap(xts[k][:], 1, 1)
                    t12 = _tap_ap(xts[k][:], 1, 2)
                    t21 = _tap_ap(xts[k][:], 2, 1)
                    t22 = _tap_ap(xts[k][:], 2, 2)
                    with nc.allow_low_precision(reason="bf16 avgpool"):
                        nc.gpsimd.tensor_tensor(out=pv, in0=t11, in1=t12,
                                                op=ALU.add)
                        nc.gpsimd.tensor_tensor(out=pv, in0=pv, in1=t21,
                                                op=ALU.add)
                        nc.gpsimd.tensor_tensor(out=pv, in0=pv, in1=t22,
                                                op=ALU.add)

                # ---- attention scores: a[l, n] (n = h*81 + p*9 + q) ----
                pa0 = ps_a.tile([128, 512], F32, tag="pa0", name="pa0")
                pa1 = ps_a.tile([128, 512], F32, tag="pa1", name="pa1")
                pa_parts = [(pa0[:, :512], slice(0, 512)),
                            (pa1[:, :N_ATT - 512], slice(512, N_ATT))]
                for k in range(CCH):
                    for pap, nsl in pa_parts:
                        nc.tensor.matmul(
                            pap, pooled[k][:], aw_sb[k][:, nsl],
                            start=(k == 0), stop=False)
                for pap, nsl in pa_parts:
                    nc.tensor.matmul(pap, ones_l[:], ab_sb[:, nsl],
                                     start=False, stop=True)

                # ---- softmax over q (logits tiny, skip max-subtract) ----
                att = spool.tile([128, N_ATT], BF16, tag="att")
                nc.scalar.activation(out=att[:, :512], in_=pa0[:, :512],
                                     func=AF.Exp)
                nc.scalar.activation(out=att[:, 512:], in_=pa1[:, :N_ATT - 512],
                                     func=AF.Exp)
                den = spool.tile([128, 108], BF16, tag="den")
                denr = spool.tile([128, 108], BF16, tag="denr")
                with nc.allow_low_precision(reason="softmax denom in bf16"):
                    nc.vector.tensor_reduce(
                        out=den[:],
                        in_=att[:].rearrange("p (g q) -> p g q", q=K2),
                        axis=AX.X, op=ALU.add)
                    nc.vector.reciprocal(out=denr[:], in_=den[:])
                nc.vector.tensor_tensor(
                    out=att[:].rearrange("p (g q) -> p g q", q=K2),
                    in0=att[:].rearrange("p (g q) -> p g q", q=K2),
                    in1=denr[:].unsqueeze(2).broadcast_to([128, 108, K2]),
                    op=ALU.mult)

                # ---- grouped conv: u[l, (h, d, q)] ----
                # matmul weight APs must merge to one free dim, so the
                # (i, j) window view is fed as 4 concurrent col-tiles
                # (one per window row, tile_position on 32-col groups)
                u_sb = upool.tile([128, K2 * C], BF16, tag="u")
                uq = u_sb[:].rearrange("p (h d q) -> p q h d",
                                       h=HEADS, d=HD, q=K2)
                for q in range(K2):
                    qi, qj = q // K, q % K
                    pu = ps_u.tile([128, 512], F32, tag="pu", name="pu")
                    for k in range(CCH):
                        for i in range(4):
                            row = bass.AP(
                                xts[k].tensor,
                                xts[k][:].offset + (2 * i + qi) * GW + qj,
                                [list(xts[k][:].ap[0]), [2, 32]])
                            nc.tensor.matmul(
                                pu[32 * i:32 * (i + 1), :C], row,
                                cw_sb[k][:, q * C:(q + 1) * C],
                                start=(k == 0), stop=(k == CCH - 1),
                                tile_position=(0, 32 * i))
                    # interleaved (h, d, q) store: strided writes run at
                    # 1x everywhere; keep them on ACT (DVE is the
                    # critical path)
                    nc.scalar.copy(
                        out=uq[:, q],
                        in_=pu[:, :C].rearrange("p (h d) -> p h d", h=HEADS))
                return att, u_sb

            def _body(lc, att, u_sb):
                # ---- apply + q-summed transpose + parity fold ----
                # parity classes: rows E (y even <- pi=1) / O (y odd <-
                # pi in {0,2}); cols E (x even <- pj=1) / O (pj in {0,2}).
                # O-row accumulators have 5 rows (160 cols); row 0 is the
                # chunk-boundary row shared with the previous lc.
                # T1 bank: ee [0:128], eo [128:256], oe [256:416]
                # T2 bank: oo [0:160]
                # A zero dummy matmul (start=True) covers each bank's full
                # used range first: clears has_written for the whole bank,
                # writes 0, and WAW-orders every later in-range matmul
                # after it. All real matmuls then accumulate (start=False).
                for k in range(CCH):
                    pc1 = ps_c.tile([128, 512], F32, tag="pc1", name="pc1")
                    pc2 = ps_c.tile([128, 512], F32, tag="pc2", name="pc2")
                    nc.tensor.matmul(pc1[:, 0:416], ident_bf[:],
                                     zeros_bf[:, 0:416], start=True, stop=False)
                    nc.tensor.matmul(pc2[:, 0:160], ident_bf[:],
                                     zeros_bf[:, 0:160], start=True, stop=False)
                    n1 = n2 = 0  # real-MM counters for stop flags

                    h0 = k * HCH
                    uv = u_sb[:].rearrange("p (h d q) -> p h d q",
                                           h=HEADS, d=HD, q=K2)
                    attv = att[:].rearrange("p (h pp q) -> p pp h q",
                                            h=HEADS, pp=K2)
                    for p in range(K2):
                        pi, pj = p // K, p % K
                        tt = tpool.tile([128, K2 * 128], BF16, tag="tt")
                        ttv = tt[:].rearrange("p (h d q) -> p h d q",
                                              h=HCH, d=HD, q=K2)
                        with nc.allow_low_precision(reason="bf16 apply"):
                            nc.vector.tensor_tensor(
                                out=ttv,
                                in0=uv[:, h0:h0 + HCH, :, :],
                                in1=attv[:, p, h0:h0 + HCH].unsqueeze(2)
                                    .broadcast_to([128, HCH, HD, K2]),
                                op=ALU.mult)

                        if pi == 1:
                            coff = 0 if pj == 1 else 128    # ee / eo
                        else:
                            # odd rows: r' = i for pi=0, i+1 for pi=2
                            coff = 256 + (32 if pi == 2 else 0)  # oe
                        use2 = (pi != 1) and (pj != 1)           # oo
                        if use2:
                            ptile, coff = pc2, (32 if pi == 2 else 0)
                        else:
                            ptile = pc1
                        rhs = shift_bf if pj == 0 else ident_bf[:]
                        ttq = tt[:].rearrange("p (h d q) -> p q h d",
                                              h=HCH, d=HD, q=K2)
                        for q in range(K2):
                            if use2:
                                n2 += 1
                                sp = n2 == 36
                            else:
                                n1 += 1
                                sp = n1 == 45
                            nc.tensor.matmul(
                                ptile[:, coff:coff + 128],
                                ttq[:, q], rhs,
                                start=False, stop=sp)

                    # ---- evacuate parity classes into the bf16 grid ----
                    gt = grid[k]
                    base_e = (8 * lc + 1) * GW   # first even row (y=8lc)
                    base_o1 = (8 * lc + 2) * GW  # odd rows y=8lc+1..
                    base_o0 = (8 * lc) * GW      # boundary row y=8lc-1

                    def _gview(off, nrows):
                        return bass.AP(gt.tensor, gt[:].offset + off,
                                       [list(gt[:].ap[0]),
                                        [2 * GW, nrows], [2, 32]])

                    nc.scalar.copy(out=_gview(base_e + 1, 4),
                                   in_=pc1[:, 0:128].rearrange(
                                       "p (r j) -> p r j", r=4))
                    nc.scalar.copy(out=_gview(base_e + 2, 4),
                                   in_=pc1[:, 128:256].rearrange(
                                       "p (r j) -> p r j", r=4))
                    nc.scalar.copy(out=_gview(base_o1 + 1, 4),
                                   in_=pc1[:, 288:416].rearrange(
                                       "p (r j) -> p r j", r=4))
                    nc.scalar.copy(out=_gview(base_o1 + 2, 4),
                                   in_=pc2[:, 32:160].rearrange(
                                       "p (r j) -> p r j", r=4))
                    with nc.allow_low_precision(reason="bf16 boundary add"):
                        nc.vector.tensor_tensor(
                            out=_gview(base_o0 + 1, 1), in0=_gview(base_o0 + 1, 1),
                            in1=pc1[:, 256:288].rearrange("p (r j) -> p r j", r=1),
                            op=ALU.add)
                        nc.vector.tensor_tensor(
                            out=_gview(base_o0 + 2, 1), in0=_gview(base_o0 + 2, 1),
                            in1=pc2[:, 0:32].rearrange("p (r j) -> p r j", r=1),
                            op=ALU.add)

            # ---- main loop over l-chunks (pipelined) ----
            proj_done = 0
            _slab_load(1)
            stage = _head(0)
            for lc in range(NLC):
                if lc + 1 < NLC:
                    if lc + 2 < NLC:
                        _slab_load(lc + 2)
                    nxt = _head(lc + 1)
                else:
                    nxt = None
                _body(lc, *stage)
                stage = nxt
                # projection stripes whose rows are now final
                ready = 10 if lc == NLC - 1 else min(9, (8 * lc) // 7) + 1
                while proj_done < ready:
                    _proj_stripe(proj_done)
                    proj_done += 1

    if not nc.is_finalized():
        nc.finalize()
    return nc


_NC_CACHE = None


def _get_nc():
    global _NC_CACHE
    if _NC_CACHE is None:
        _NC_CACHE = _build()
    return _NC_CACHE


def _bf16(a):
    import ml_dtypes
    return np.ascontiguousarray(a.astype(ml_dtypes.bfloat16))


def _prep_weights(attn_w, attn_b, conv_w, proj_w, proj_b):
    scale = (C // HEADS) ** -0.5
    aw = (attn_w.astype(np.float64) * scale * 0.25).astype(np.float32)
    aw_t = _bf16(aw.T)                                                   # [C, 972]
    ab = _bf16((attn_b * scale).astype(np.float32).reshape(1, N_ATT))
    cw = conv_w.reshape(K2, C, C).transpose(2, 0, 1).reshape(C, K2 * C)  # [c_in, (q, c_out)]
    cw = _bf16(cw.astype(np.float32))
    pw_t = _bf16(proj_w.astype(np.float32).T)                            # [c_in, c_out]
    pb = np.ascontiguousarray(proj_b.astype(np.float32).reshape(C, 1))
    return aw_t, ab, cw, pw_t, pb


_EXEC = None     # jitted shard_map executable + shardings
_WDEV = None     # (host weight arrays, device weight arrays) cache
_YBUF = None     # previous output device buffer, reused as donated output


def _make_exec(nc):
    import jax
    import jax.numpy as jnp
    from jax.experimental.shard_map import shard_map
    from jax.sharding import Mesh, NamedSharding, PartitionSpec as PSpec
    from concourse import bass2jax
    import concourse.mybir as mybir_

    bass2jax.install_neuronx_cc_hook()
    partition_name = (nc.partition_id_tensor.name
                      if nc.partition_id_tensor else None)
    in_names, out_names, out_avals = [], [], []
    for alloc in nc.m.functions[0].allocations:
        if not isinstance(alloc, mybir_.MemoryLocationSet):
            continue
        name = alloc.memorylocations[0].name
        if alloc.kind == "ExternalInput":
            if name != partition_name:
                in_names.append(name)
        elif alloc.kind == "ExternalOutput":
            out_names.append(name)
            out_avals.append(jax.core.ShapedArray(
                tuple(alloc.tensor_shape), mybir_.dt.np(alloc.dtype)))
    n_params = len(in_names)
    all_in = list(in_names) + list(out_names)
    if partition_name is not None:
        all_in.append(partition_name)

    def _body(*args):
        operands = list(args)
        if partition_name is not None:
            operands.append(bass2jax.partition_id_tensor())
        outs = bass2jax._bass_exec_p.bind(
            *operands, out_avals=tuple(out_avals), in_names=tuple(all_in),
            out_names=tuple(out_names), lowering_input_output_aliases=(),
            sim_require_finite=True, sim_require_nnan=True, nc=nc)
        return tuple(outs)

    devices = jax.devices()[:NCORES]
    mesh = Mesh(np.asarray(devices), ("core",))
    percore = {"x"}
    specs = [PSpec("core") if nm in percore else PSpec() for nm in in_names]
    specs += [PSpec("core")] * len(out_names)
    donate = tuple(range(n_params, n_params + len(out_names)))
    sharded = jax.jit(
        shard_map(_body, mesh=mesh, in_specs=tuple(specs),
                  out_specs=(PSpec("core"),) * len(out_names),
                  check_rep=False),
        donate_argnums=donate, keep_unused=True)
    zsh = NamedSharding(mesh, PSpec("core"))
    zshapes = [((NCORES * av.shape[0],) + tuple(av.shape[1:]), av.dtype)
               for av in out_avals]
    zjit = jax.jit(lambda: tuple(jnp.zeros(s, d) for s, d in zshapes),
                   out_shardings=(zsh,) * len(out_names))
    return {"sharded": sharded, "zjit": zjit,
            "xsh": NamedSharding(mesh, PSpec("core")),
            "wsh": NamedSharding(mesh, PSpec()),
            "in_names": in_names, "out_names": out_names}


def _weights_dev(wmap, E):
    """Device-put the (replicated) weights; reuse cached device arrays
    when the host contents are unchanged between calls."""
    global _WDEV
    import jax
    if _WDEV is not None:
        host, dev = _WDEV
        if all(np.array_equal(host[k], wmap[k]) for k in wmap):
            return dev
    dev = {k: jax.device_put(v, E["wsh"]) for k, v in wmap.items()}
    jax.block_until_ready(list(dev.values()))
    _WDEV = ({k: v.copy() for k, v in wmap.items()}, dev)
    return dev


def _run_fast(x_bf, wmap, E, timing=None):
    import time
    import jax
    global _YBUF
    wdev = _weights_dev(wmap, E)
    t0 = time.perf_counter()
    xdev = jax.device_put(x_bf, E["xsh"])
    jax.block_until_ready(xdev)
    t1 = time.perf_counter()
    if _YBUF is not None:
        # y is fully overwritten by the kernel; donate last call's output
        # buffer instead of dispatching a fresh device-side zeros fill.
        zeros = (_YBUF,)
        _YBUF = None
    else:
        zeros = E["zjit"]()
        jax.block_until_ready(zeros)
    inmap = {"x": xdev, **wdev}
    args = [inmap[nm] for nm in E["in_names"]]
    t2 = time.perf_counter()
    out = E["sharded"](*args, *zeros)
    jax.block_until_ready(out)
    t3 = time.perf_counter()
    y = np.asarray(out[0])
    _YBUF = out[0]
    t4 = time.perf_counter()
    if timing is not None:
        timing.append({"x_put": t1 - t0, "zeros": t2 - t1,
                       "exec": t3 - t2, "fetch": t4 - t3})
    return y


def kernel(x, attn_w, attn_b, conv_w, proj_w, proj_b, _trace=False, _dbg=False):
    global _EXEC
    x = np.asarray(x, dtype=np.float32)
    aw_t, ab, cw, pw_t, pb = _prep_weights(
        np.asarray(attn_w), np.asarray(attn_b), np.asarray(conv_w),
        np.asarray(proj_w), np.asarray(proj_b))
    wmap = {"aw": aw_t, "ab": ab, "cw": cw, "pw": pw_t, "pb": pb}
    x_bf = _bf16(x.reshape(NCORES * C, H * W))
    nc = _get_nc()
    try:
        if _EXEC is None:
            _EXEC = _make_exec(nc)
        y2d = _run_fast(x_bf, wmap, _EXEC, timing=None)
    except Exception as e:
        print(f"fast path failed ({type(e).__name__}: {e}); using legacy",
              file=sys.stderr)
        from concourse.bass_utils import run_bass_kernel_spmd
        in_maps = []
        for b in range(NCORES):
            in_maps.append({
                "x": np.ascontiguousarray(x_bf[b * C:(b + 1) * C]),
                **wmap,
            })
        res = run_bass_kernel_spmd(nc, in_maps, list(range(NCORES)),
                                   trace=False)
        y2d = np.concatenate([np.asarray(res.results[b]["y"])
                              for b in range(NCORES)], axis=0)
    return y2d.astype(np.float32).reshape(B, C, H, W)
